# revision 39
# baseline (speedup 1.0000x reference)
"""Causal self-attention (B=4, T=2048, C=1024, NH=16) on 8 trn2 NeuronCores.

Sharding: core = (head_group hg in {0,1}) x (batch b in {0..3}).
Each core computes qkv projection + attention + partial output projection for
its 8 heads of its batch; host sums the two head-group partials per batch and
adds the output bias (plus the v-bias contribution, which is linear in Wproj
and therefore folded into a host-side constant: P(V + 1 bv^T)/d = PV/d + bv).

Layout strategy (all matmuls in bf16: 1 PE cycle/row at ANY moving size,
vs fp32r which needs >=256; rel err ~5e-3, well inside the 2e-2 gate):
  - x, Wqkv, Wproj are converted to bf16 on the host; everything stays
    resident in SBUF (no DRAM staging round-trips).
  - q, k computed transposed (qT/kT = W_slice @ x.T) so head_size lands on
    partitions for QK^T.  v is computed in NATURAL [token, feat] layout
    (lhsT = xT tile, rhs = Wv) so no PE transposes of V are needed.
  - S^T = K @ Q^T per (head, 4-keytile group) into one [128, 1024] PSUM
    2-bank tile; one ScalarE Exp per group (amortizes the ~190ns ACT per-op
    overhead); causal mask = one 0/1 multiply on GpSimd over the two
    diagonal key tiles (always within one group).
  - AV runs in natural orientation: out[128 q, 65] with rhs = [V_head | 1];
    the ones column makes the softmax denominator ride along as column 64
    (65 moving columns instead of 256 -> AV PE cost halves vs transposed).
    The two 128-query halves are processed sequentially per head so 8
    accumulators (4 aligned head slots x 128 cols) fit one PSUM bank each.
  - 1/d is applied as the PSUM->SBUF copy itself (per-partition
    tensor_scalar_mul), then one bf16 PE transpose per (head pair, q half)
    produces O^T for the row-parallel output projection.
  - The Tile scheduler is a greedy per-engine priority list scheduler
    (priority = emission order) and dependencies follow emission-order
    semantics (a read emitted before a write sees the old value).  The
    projection is therefore emitted as 256-column units flushed per-pair
    with one-pair lookahead inside the qtile that consumes them: each unit
    executes while the previous pair's attention streams, the exp conveyor
    never waits on a whole-chunk backlog at qtile boundaries, and in the
    exp(softmax)-heavy late qtiles the units are the dep-free PE filler.
    The output projections of qtiles 4-6 are deferred into qtiles 6/7
    (their only consumer is the output DMA) as additional late filler,
    with their PSUM tiles in the feed tag so they do not chain behind the
    attention pipeline's slot rotation.
  - PSUM budget (8 banks): sg (exp staging) 4, feed-proj "ps" 2, oav
    accumulators + transposes + out-proj "oav" 2.  Feed units get their own
    tag because tag slots are granted in emission order: sharing a tag with
    the attention pipeline would chain low-priority filler behind it.
"""

import sys

sys.path.insert(0, "/opt/trn_rl_repo")

import ml_dtypes
import numpy as np

import concourse.bacc as bacc
import concourse.bass as bass
import concourse.mybir as mybir
from concourse.bass_utils import run_bass_kernel_spmd
from concourse.masks import make_identity
from concourse.tile import TileContext

B, T, C, NH = 4, 2048, 1024, 16
HS = C // NH          # 64
HGF = 512             # features per head group (8 heads x 64)
QT = 256              # query tile
NKT = T // 128        # 16 key tiles
F32 = mybir.dt.float32
BF16 = mybir.dt.bfloat16
Exp = mybir.ActivationFunctionType.Exp
BF = ml_dtypes.bfloat16

KPOS = [1, 5, 7, 9]   # position of k_p within a 10-unit half-chunk feed


def build_kernel():
    nc = bacc.Bacc(None, target_bir_lowering=False)
    xT = nc.dram_tensor("xT", (C, T), BF16, kind="ExternalInput")
    wqkvT = nc.dram_tensor("wqkvT", (C, 3 * HGF), BF16, kind="ExternalInput")
    bqk = nc.dram_tensor("bqk", (128, 8), F32, kind="ExternalInput")
    wprojT = nc.dram_tensor("wprojT", (HGF, C), BF16, kind="ExternalInput")
    mask01 = nc.dram_tensor("mask01", (128, 384), BF16, kind="ExternalInput")
    y = nc.dram_tensor("y", (T, C), F32, kind="ExternalOutput")

    with TileContext(nc) as tc:
        with (
            tc.tile_pool(name="outer", bufs=1) as outer,
            tc.tile_pool(name="work", bufs=1) as work,
            tc.tile_pool(name="psum", bufs=1, space="PSUM") as psum,
        ):
            identf = outer.tile([128, 128], F32, name="identf")
            make_identity(nc, identf)
            ident = outer.tile([128, 128], BF16, name="ident")
            nc.vector.tensor_copy(ident, identf)

            x_all = outer.tile([128, 8 * T], BF16, name="x_all")
            w_all = outer.tile([128, 8 * 1536], BF16, name="w_all")
            q_sb = [outer.tile([128, T], BF16, name=f"q{p}") for p in range(4)]
            k_sb = [outer.tile([128, T], BF16, name=f"k{p}") for p in range(4)]
            # v natural [token 128, 8 heads x (64 data + 1 ones)]
            vnat = [outer.tile([128, 520], BF16, name=f"v{i}") for i in range(NKT)]
            wp_sb = [outer.tile([128, C], BF16, name=f"wp{p}") for p in range(4)]
            mask_b = outer.tile([128, 384], BF16, name="mask_b")
            bqk_sb = outer.tile([128, 8], F32, name="bqk_sb")

            # denominator ones columns; v copies only touch the data columns
            for i in range(NKT):
                nc.vector.memset(
                    vnat[i].rearrange("p (h c) -> p h c", c=65)[:, :, 64:65], 1.0
                )

            # loads as wide single-DMA waves (HWDGE + the DMA engine pool
            # are single-slot in the cost model: few big transfers beat many
            # small ones).  Column order = first need: w q-cols, x chunk 0,
            # w k-cols, w v-cols, then the remaining x chunks.
            w_dst = w_all.rearrange("p (k f) -> p k f", f=1536)
            w_src = wqkvT.rearrange("(k p) f -> p k f", p=128)
            x_dst = x_all.rearrange("p (k t) -> p k t", t=T)
            x_src = xT.rearrange("(k p) t -> p k t", p=128)
            nc.sync.dma_start(w_dst[:, :, 0:128], w_src[:, :, 0:128])
            nc.scalar.dma_start(x_dst[:, :, 0:256], x_src[:, :, 0:256])
            nc.gpsimd.dma_start(bqk_sb, bqk[:, :])
            nc.gpsimd.dma_start(mask_b, mask01[:, :])
            nc.sync.dma_start(w_dst[:, :, 512:640], w_src[:, :, 512:640])
            nc.scalar.dma_start(w_dst[:, :, 1024:1536], w_src[:, :, 1024:1536])
            nc.scalar.dma_start(x_dst[:, :, 256:512], x_src[:, :, 256:512])
            nc.sync.dma_start(w_dst[:, :, 128:512], w_src[:, :, 128:512])
            nc.sync.dma_start(w_dst[:, :, 640:1024], w_src[:, :, 640:1024])
            nc.sync.dma_start(x_dst[:, :, 512:1024], x_src[:, :, 512:1024])
            nc.scalar.dma_start(x_dst[:, :, 1024:1536], x_src[:, :, 1024:1536])
            nc.sync.dma_start(x_dst[:, :, 1536:2048], x_src[:, :, 1536:2048])
            for p in range(4):
                (nc.gpsimd if p % 2 == 0 else nc.scalar).dma_start(
                    wp_sb[p], wprojT[p * 128:(p + 1) * 128, :])

            def emit_qk(n, m, half):
                c0 = n * 512 + half * 256
                ps = psum.tile([128, 256], F32, tag="ps", bufs=2,
                               name=f"ps{n}_{m}_{half}")
                for k in range(8):
                    nc.tensor.matmul(
                        ps,
                        w_all[:, k * 1536 + m * 128:k * 1536 + (m + 1) * 128],
                        x_all[:, k * T + c0:k * T + c0 + 256],
                        start=(k == 0),
                        stop=(k == 7),
                    )
                dst = q_sb[m] if m < 4 else k_sb[m - 4]
                nc.vector.tensor_scalar_add(
                    dst[:, c0:c0 + 256], ps, bqk_sb[:, m:m + 1]
                )

            def emit_v(n, t4):
                tk = 4 * n + t4
                psv = psum.tile([128, 512], F32, tag="ps", bufs=2,
                                name=f"psv{tk}")
                for k in range(8):
                    nc.tensor.matmul(
                        psv,
                        x_all[:, k * T + tk * 128:k * T + (tk + 1) * 128],
                        w_all[:, k * 1536 + 1024:(k + 1) * 1536],
                        start=(k == 0),
                        stop=(k == 7),
                    )
                nc.vector.tensor_copy(
                    vnat[tk].rearrange("p (h c) -> p h c", c=65)[:, :, 0:64],
                    psv.rearrange("p (h c) -> p h c", c=64),
                )

            # feed units per chunk: a = first 256 tokens (needed by
            # qtile 2c), bq = q of second 256 (needed at qtile 2c+1 start),
            # bkv = k/v of second 256 (needed only by qtile 2c+1's LAST
            # QK/AV groups - legal to emit at lowest priority so the list
            # scheduler pulls them into PE's exp-wait stalls).
            units = []
            for n in range(4):
                a, bq, bkv = [], [], []
                for p in range(4):
                    a.append(lambda n=n, m=p: emit_qk(n, m, 0))
                    a.append(lambda n=n, m=p: emit_qk(n, 4 + m, 0))
                    bq.append(lambda n=n, m=p: emit_qk(n, m, 1))
                    bkv.append(lambda n=n, m=p: emit_qk(n, 4 + m, 1))
                a.insert(2, lambda n=n: emit_v(n, 0))
                a.insert(3, lambda n=n: emit_v(n, 1))
                bkv.append(lambda n=n: emit_v(n, 2))
                bkv.append(lambda n=n: emit_v(n, 3))
                units.append({"a": a, "bq": bq, "bkv": bkv})
            # per-chunk queues in pair-need order; flushed per-pair with
            # one-pair lookahead so each unit executes while the previous
            # pair's attention streams, and the exp stream never waits for
            # a whole-chunk backlog at qtile boundaries.
            AQ = [u["a"] for u in units]
            BQ = [[u["bq"][0], u["bkv"][0], u["bkv"][4], u["bkv"][5],
                   u["bq"][1], u["bkv"][1], u["bq"][2], u["bkv"][2],
                   u["bq"][3], u["bkv"][3]] for u in units]
            CUM = [4, 6, 8, 10]
            fpos = {}

            def flush(j, pair):
                ch = j // 2
                q = AQ[ch] if j % 2 == 0 else BQ[ch]
                key = (ch, j % 2)
                limit = CUM[pair]
                while fpos.get(key, 0) < limit:
                    q[fpos.get(key, 0)]()
                    fpos[key] = fpos.get(key, 0) + 1

            def extract_muls(j, pg, oav, opairs, on_act=False):
                # 1/d for 4 heads x 2 q-halves; all PSUM reads up front so
                # the oav slots release before any transpose allocates in
                # the shared "oav" psum tag rotation.
                for hf in range(2):
                    dinv = work.tile([128, 4], F32, tag="dinv", bufs=4,
                                     name=f"dinv{j}_{pg}_{hf}")
                    nc.vector.reciprocal(
                        dinv,
                        oav[hf].rearrange(
                            "p (s c) -> p s c", c=128)[:, :, 64:65],
                    )
                    for pi, pr in enumerate((2 * pg, 2 * pg + 1)):
                        opair = work.tile([128, 128], BF16, tag="opair",
                                          bufs=5, name=f"op{j}_{hf}_{pr}")
                        for s in range(2):
                            sl = (2 * pr + s) % 4
                            if on_act:
                                nc.scalar.activation(
                                    opair[:, s * 64:(s + 1) * 64],
                                    oav[hf][:, sl * 128:sl * 128 + 64],
                                    mybir.ActivationFunctionType.Identity,
                                    scale=dinv[:, sl:sl + 1],
                                )
                            else:
                                nc.vector.tensor_scalar_mul(
                                    opair[:, s * 64:(s + 1) * 64],
                                    oav[hf][:, sl * 128:sl * 128 + 64],
                                    dinv[:, sl:sl + 1],
                                )
                        opairs[hf][pi] = opair

            def extract_tr(j, pg, hf, opairs, o_j):
                for pi, pr in enumerate((2 * pg, 2 * pg + 1)):
                    ot = psum.tile([128, 128], BF16, tag="oav", bufs=2,
                                   name=f"ot{j}_{hf}_{pr}")
                    nc.tensor.transpose(ot, opairs[hf][pi], ident)
                    nc.vector.tensor_copy(
                        o_j[pr][:, hf * 128:(hf + 1) * 128], ot
                    )

            def outproj(j, mm, o_j, tail=False, ptag="oav"):
                jq = j * QT
                ysb = work.tile([128, C], F32, tag="ysb", bufs=3,
                                name=f"ys{j}_{mm}")
                for nn in range(2):
                    psy = psum.tile([128, 512], F32, tag=ptag, bufs=2,
                                    name=f"py{j}_{mm}_{nn}")
                    for p in range(4):
                        nc.tensor.matmul(
                            psy,
                            o_j[p][:, mm * 128:(mm + 1) * 128],
                            wp_sb[p][:, nn * 512:(nn + 1) * 512],
                            start=(p == 0),
                            stop=(p == 3),
                        )
                    if tail and nn == 1:
                        nc.scalar.activation(
                            ysb[:, nn * 512:(nn + 1) * 512], psy,
                            mybir.ActivationFunctionType.Identity)
                    else:
                        nc.vector.tensor_copy(
                            ysb[:, nn * 512:(nn + 1) * 512], psy)
                    nc.sync.dma_start(
                        y[jq + mm * 128:jq + (mm + 1) * 128,
                          nn * 512:(nn + 1) * 512],
                        ysb[:, nn * 512:(nn + 1) * 512],
                    )

            # qtile 0 pair 0/1's dependencies run before any attention
            flush(0, 1)
            deferred = []

            for j in range(8):
                ch = j // 2
                jq = j * QT
                ntk = 2 * (j + 1)
                ngrp = (ntk + 3) // 4
                o_j = [
                    work.tile([128, QT], BF16, tag=f"oj{p}", bufs=4,
                              name=f"o{p}_{j}")
                    for p in range(4)
                ]
                opairs = [[None, None], [None, None]]
                for pg in range(2):
                    oav = [
                        psum.tile([128, 512], F32, tag="oav", bufs=2,
                                  name=f"oav{j}_{pg}_{hf}")
                        for hf in range(2)
                    ]
                    for pair in (2 * pg, 2 * pg + 1):
                        for s in range(2):
                            h = 2 * pair + s
                            off = 64 * s
                            hslot = h % 4
                            pts = []
                            for g in range(ngrp):
                                blk = min(4, ntk - 4 * g)
                                diag = g == j // 2
                                # last key tile 2j+1: queries 0:127 are fully
                                # masked - compute only the valid q half
                                cols = blk * QT - (128 if diag else 0)
                                sg = psum.tile([128, 1024], F32, tag="sg",
                                               bufs=2, name=f"sg{j}_{h}_{g}")
                                for bi in range(blk):
                                    i = 4 * g + bi
                                    if diag and i == ntk - 1:
                                        nc.tensor.matmul(
                                            sg[:, bi * QT:bi * QT + 128],
                                            k_sb[pair][off:off + 64,
                                                       i * 128:(i + 1) * 128],
                                            q_sb[pair][off:off + 64,
                                                       jq + 128:jq + QT],
                                            start=True,
                                            stop=True,
                                        )
                                    else:
                                        nc.tensor.matmul(
                                            sg[:, bi * QT:(bi + 1) * QT],
                                            k_sb[pair][off:off + 64,
                                                       i * 128:(i + 1) * 128],
                                            q_sb[pair][off:off + 64, jq:jq + QT],
                                            start=True,
                                            stop=True,
                                        )
                                pt = work.tile([128, 1024], BF16, tag="pt",
                                               bufs=5, name=f"pt{j}_{h}_{g}")
                                nc.scalar.activation(
                                    pt[:, :cols], sg[:, :cols], Exp,
                                    scale=0.125)
                                if diag:  # diagonal key tiles 2j, 2j+1
                                    pos = (blk - 2) * QT
                                    nc.gpsimd.tensor_mul(
                                        pt[:, pos:pos + 384],
                                        pt[:, pos:pos + 384], mask_b)
                                pts.append(pt)
                                for bi in range(blk):  # q half 0
                                    i = 4 * g + bi
                                    if i == ntk - 1:
                                        continue  # fully masked for q half 0
                                    nc.tensor.matmul(
                                        oav[0][:, hslot * 128:hslot * 128 + 65],
                                        pt[:, bi * QT:bi * QT + 128],
                                        vnat[i][:, h * 65:h * 65 + 65],
                                        start=(i == 0),
                                        stop=(i == ntk - 2),
                                    )
                            for g in range(ngrp):  # q half 1
                                blk = min(4, ntk - 4 * g)
                                for bi in range(blk):
                                    i = 4 * g + bi
                                    lo = bi * QT + (0 if i == ntk - 1 else 128)
                                    nc.tensor.matmul(
                                        oav[1][:, hslot * 128:hslot * 128 + 65],
                                        pts[g][:, lo:lo + 128],
                                        vnat[i][:, h * 65:h * 65 + 65],
                                        start=(i == 0),
                                        stop=(i == ntk - 1),
                                    )
                        if pair < 3:
                            flush(j, pair + 1)
                        elif j < 7:
                            flush(j + 1, 1)
                    extract_muls(j, pg, oav, opairs,
                                 on_act=(j == 7 and pg == 1))
                    if pg == 0:
                        extract_tr(j, 0, 0, opairs, o_j)
                        extract_tr(j, 0, 1, opairs, o_j)
                if j in (4, 5, 6):
                    # defer this qtile's output projection into the next
                    # qtile's exp-heavy window as extra PE filler
                    extract_tr(j, 1, 0, opairs, o_j)
                    extract_tr(j, 1, 1, opairs, o_j)
                    deferred.append((j, o_j))
                else:
                    # interleave last extractions with the output projection
                    extract_tr(j, 1, 0, opairs, o_j)
                    outproj(j, 0, o_j, tail=(j == 7))
                    extract_tr(j, 1, 1, opairs, o_j)
                    outproj(j, 1, o_j, tail=(j == 7))
                for dj, do_j in list(deferred):
                    if (j == 6 and dj in (4, 5)) or (j == 7 and dj == 6):
                        deferred.remove((dj, do_j))
                        outproj(dj, 0, do_j, tail=(j == 7), ptag="ps")
                        outproj(dj, 1, do_j, tail=(j == 7), ptag="ps")

    nc.finalize()
    return nc


_NC = None


def _get_nc():
    global _NC
    if _NC is None:
        _NC = build_kernel()
    return _NC


def kernel(x, Wqkv, bqkv, Wproj, bproj, _trace=False):
    x = np.asarray(x, dtype=np.float32)
    Wqkv = np.asarray(Wqkv, dtype=np.float32)
    bqkv = np.asarray(bqkv, dtype=np.float32)
    Wproj = np.asarray(Wproj, dtype=np.float32)
    bproj = np.asarray(bproj, dtype=np.float32)

    tri = np.triu(np.ones((2 * QT, 2 * QT), dtype=np.float32))[:, :QT]
    mask = np.ascontiguousarray(np.concatenate(
        [tri[0:128, 0:QT], tri[128:256, 128:QT]], axis=1)).astype(BF)
    in_maps = []
    for hg in range(2):
        sl = slice(hg * HGF, (hg + 1) * HGF)
        rows = np.concatenate([
            Wqkv[sl],
            Wqkv[1024 + hg * HGF:1024 + (hg + 1) * HGF],
            Wqkv[2048 + hg * HGF:2048 + (hg + 1) * HGF],
        ])
        wqkvT_h = np.ascontiguousarray(rows.T).astype(BF)      # [C, 1536]
        bq = bqkv[sl].reshape(4, 128).T
        bk = bqkv[1024 + hg * HGF:1024 + (hg + 1) * HGF].reshape(4, 128).T
        bqk_h = np.ascontiguousarray(
            np.concatenate([bq, bk], axis=1), dtype=np.float32)  # [128, 8]
        wprojT_h = np.ascontiguousarray(Wproj[:, sl].T).astype(BF)  # [512, C]
        for b in range(B):
            in_maps.append(
                {
                    "xT": np.ascontiguousarray(x[b].T).astype(BF),
                    "wqkvT": wqkvT_h,
                    "bqk": bqk_h,
                    "wprojT": wprojT_h,
                    "mask01": mask,
                }
            )
    # core order: idx = hg * 4 + b
    res = run_bass_kernel_spmd(_get_nc(), in_maps, core_ids=list(range(8)),
                               trace=_trace)
    # v-bias contribution is linear: folded into one host-side constant
    yconst = (bproj + Wproj @ bqkv[2048:]).astype(np.float32)
    out = np.empty((B, T, C), dtype=np.float32)
    for b in range(B):
        out[b] = res.results[b]["y"] + res.results[4 + b]["y"] + yconst
    if _trace:
        return out, res
    return out


# revision 40
# speedup vs baseline: 1.0008x; 1.0008x over previous
"""Causal self-attention (B=4, T=2048, C=1024, NH=16) on 8 trn2 NeuronCores.

Sharding: core = (head_group hg in {0,1}) x (batch b in {0..3}).
Each core computes qkv projection + attention + partial output projection for
its 8 heads of its batch; host sums the two head-group partials per batch and
adds the output bias (plus the v-bias contribution, which is linear in Wproj
and therefore folded into a host-side constant: P(V + 1 bv^T)/d = PV/d + bv).

Layout strategy (all matmuls in bf16: 1 PE cycle/row at ANY moving size,
vs fp32r which needs >=256; rel err ~5e-3, well inside the 2e-2 gate):
  - x, Wqkv, Wproj are converted to bf16 on the host; everything stays
    resident in SBUF (no DRAM staging round-trips).
  - q, k computed transposed (qT/kT = W_slice @ x.T) so head_size lands on
    partitions for QK^T.  v is computed in NATURAL [token, feat] layout
    (lhsT = xT tile, rhs = Wv) so no PE transposes of V are needed.
  - S^T = K @ Q^T per (head, 4-keytile group) into one [128, 1024] PSUM
    2-bank tile; one ScalarE Exp per group (amortizes the ~190ns ACT per-op
    overhead); causal mask = one 0/1 multiply on GpSimd over the two
    diagonal key tiles (always within one group).
  - AV runs in natural orientation: out[128 q, 65] with rhs = [V_head | 1];
    the ones column makes the softmax denominator ride along as column 64
    (65 moving columns instead of 256 -> AV PE cost halves vs transposed).
    The two 128-query halves are processed sequentially per head so 8
    accumulators (4 aligned head slots x 128 cols) fit one PSUM bank each.
  - 1/d is applied as the PSUM->SBUF copy itself (per-partition
    tensor_scalar_mul), then one bf16 PE transpose per (head pair, q half)
    produces O^T for the row-parallel output projection.
  - The Tile scheduler is a greedy per-engine priority list scheduler
    (priority = emission order) and dependencies follow emission-order
    semantics (a read emitted before a write sees the old value).  The
    projection is therefore emitted as 256-column units flushed per-pair
    with one-pair lookahead inside the qtile that consumes them: each unit
    executes while the previous pair's attention streams, the exp conveyor
    never waits on a whole-chunk backlog at qtile boundaries, and in the
    exp(softmax)-heavy late qtiles the units are the dep-free PE filler.
    The output projections of qtiles 4-6 are deferred into qtiles 6/7
    (their only consumer is the output DMA) as additional late filler,
    with their PSUM tiles in the feed tag so they do not chain behind the
    attention pipeline's slot rotation.
  - PSUM budget (8 banks): sg (exp staging) 4, feed-proj "ps" 2, oav
    accumulators + transposes + out-proj "oav" 2.  Feed units get their own
    tag because tag slots are granted in emission order: sharing a tag with
    the attention pipeline would chain low-priority filler behind it.
"""

import sys

sys.path.insert(0, "/opt/trn_rl_repo")

import ml_dtypes
import numpy as np

import concourse.bacc as bacc
import concourse.bass as bass
import concourse.mybir as mybir
from concourse.bass_utils import run_bass_kernel_spmd
from concourse.masks import make_identity
from concourse.tile import TileContext

B, T, C, NH = 4, 2048, 1024, 16
HS = C // NH          # 64
HGF = 512             # features per head group (8 heads x 64)
QT = 256              # query tile
NKT = T // 128        # 16 key tiles
F32 = mybir.dt.float32
BF16 = mybir.dt.bfloat16
Exp = mybir.ActivationFunctionType.Exp
BF = ml_dtypes.bfloat16

KPOS = [1, 5, 7, 9]   # position of k_p within a 10-unit half-chunk feed


def build_kernel():
    nc = bacc.Bacc(None, target_bir_lowering=False)
    xT = nc.dram_tensor("xT", (C, T), BF16, kind="ExternalInput")
    wqkvT = nc.dram_tensor("wqkvT", (C, 3 * HGF), BF16, kind="ExternalInput")
    bqk = nc.dram_tensor("bqk", (128, 8), F32, kind="ExternalInput")
    wprojT = nc.dram_tensor("wprojT", (HGF, C), BF16, kind="ExternalInput")
    mask01 = nc.dram_tensor("mask01", (128, 384), BF16, kind="ExternalInput")
    y = nc.dram_tensor("y", (T, C), F32, kind="ExternalOutput")

    with TileContext(nc) as tc:
        with (
            tc.tile_pool(name="outer", bufs=1) as outer,
            tc.tile_pool(name="work", bufs=1) as work,
            tc.tile_pool(name="psum", bufs=1, space="PSUM") as psum,
        ):
            identf = outer.tile([128, 128], F32, name="identf")
            make_identity(nc, identf)
            ident = outer.tile([128, 128], BF16, name="ident")
            nc.vector.tensor_copy(ident, identf)

            x_all = outer.tile([128, 8 * T], BF16, name="x_all")
            w_all = outer.tile([128, 8 * 1536], BF16, name="w_all")
            q_sb = [outer.tile([128, T], BF16, name=f"q{p}") for p in range(4)]
            k_sb = [outer.tile([128, T], BF16, name=f"k{p}") for p in range(4)]
            # v natural [token 128, 8 heads x (64 data + 1 ones)]
            vnat = [outer.tile([128, 520], BF16, name=f"v{i}") for i in range(NKT)]
            wp_sb = [outer.tile([128, C], BF16, name=f"wp{p}") for p in range(4)]
            mask_b = outer.tile([128, 384], BF16, name="mask_b")
            bqk_sb = outer.tile([128, 8], F32, name="bqk_sb")

            # denominator ones columns; v copies only touch the data columns
            for i in range(NKT):
                nc.vector.memset(
                    vnat[i].rearrange("p (h c) -> p h c", c=65)[:, :, 64:65], 1.0
                )

            # loads as wide single-DMA waves (HWDGE + the DMA engine pool
            # are single-slot in the cost model: few big transfers beat many
            # small ones).  Column order = first need: w q-cols, x chunk 0,
            # w k-cols, w v-cols, then the remaining x chunks.
            w_dst = w_all.rearrange("p (k f) -> p k f", f=1536)
            w_src = wqkvT.rearrange("(k p) f -> p k f", p=128)
            x_dst = x_all.rearrange("p (k t) -> p k t", t=T)
            x_src = xT.rearrange("(k p) t -> p k t", p=128)
            nc.sync.dma_start(w_dst[:, :, 0:128], w_src[:, :, 0:128])
            nc.scalar.dma_start(x_dst[:, :, 0:256], x_src[:, :, 0:256])
            nc.gpsimd.dma_start(bqk_sb, bqk[:, :])
            nc.gpsimd.dma_start(mask_b, mask01[:, :])
            nc.sync.dma_start(w_dst[:, :, 512:640], w_src[:, :, 512:640])
            nc.scalar.dma_start(w_dst[:, :, 1024:1536], w_src[:, :, 1024:1536])
            nc.scalar.dma_start(x_dst[:, :, 256:512], x_src[:, :, 256:512])
            nc.sync.dma_start(w_dst[:, :, 128:512], w_src[:, :, 128:512])
            nc.sync.dma_start(w_dst[:, :, 640:1024], w_src[:, :, 640:1024])
            nc.sync.dma_start(x_dst[:, :, 512:1024], x_src[:, :, 512:1024])
            nc.scalar.dma_start(x_dst[:, :, 1024:1536], x_src[:, :, 1024:1536])
            nc.sync.dma_start(x_dst[:, :, 1536:2048], x_src[:, :, 1536:2048])
            for p in range(4):
                (nc.gpsimd if p % 2 == 0 else nc.scalar).dma_start(
                    wp_sb[p], wprojT[p * 128:(p + 1) * 128, :])

            def emit_qk(n, m, half):
                c0 = n * 512 + half * 256
                ps = psum.tile([128, 256], F32, tag="ps", bufs=2,
                               name=f"ps{n}_{m}_{half}")
                for k in range(8):
                    nc.tensor.matmul(
                        ps,
                        w_all[:, k * 1536 + m * 128:k * 1536 + (m + 1) * 128],
                        x_all[:, k * T + c0:k * T + c0 + 256],
                        start=(k == 0),
                        stop=(k == 7),
                    )
                dst = q_sb[m] if m < 4 else k_sb[m - 4]
                nc.vector.tensor_scalar_add(
                    dst[:, c0:c0 + 256], ps, bqk_sb[:, m:m + 1]
                )

            def emit_v(n, t4):
                tk = 4 * n + t4
                psv = psum.tile([128, 512], F32, tag="ps", bufs=2,
                                name=f"psv{tk}")
                for k in range(8):
                    nc.tensor.matmul(
                        psv,
                        x_all[:, k * T + tk * 128:k * T + (tk + 1) * 128],
                        w_all[:, k * 1536 + 1024:(k + 1) * 1536],
                        start=(k == 0),
                        stop=(k == 7),
                    )
                nc.vector.tensor_copy(
                    vnat[tk].rearrange("p (h c) -> p h c", c=65)[:, :, 0:64],
                    psv.rearrange("p (h c) -> p h c", c=64),
                )

            # feed units per chunk: a = first 256 tokens (needed by
            # qtile 2c), bq = q of second 256 (needed at qtile 2c+1 start),
            # bkv = k/v of second 256 (needed only by qtile 2c+1's LAST
            # QK/AV groups - legal to emit at lowest priority so the list
            # scheduler pulls them into PE's exp-wait stalls).
            units = []
            for n in range(4):
                a, bq, bkv = [], [], []
                for p in range(4):
                    a.append(lambda n=n, m=p: emit_qk(n, m, 0))
                    a.append(lambda n=n, m=p: emit_qk(n, 4 + m, 0))
                    bq.append(lambda n=n, m=p: emit_qk(n, m, 1))
                    bkv.append(lambda n=n, m=p: emit_qk(n, 4 + m, 1))
                a.insert(2, lambda n=n: emit_v(n, 0))
                a.insert(3, lambda n=n: emit_v(n, 1))
                bkv.append(lambda n=n: emit_v(n, 2))
                bkv.append(lambda n=n: emit_v(n, 3))
                units.append({"a": a, "bq": bq, "bkv": bkv})
            # per-chunk queues in pair-need order; flushed per-pair with
            # one-pair lookahead so each unit executes while the previous
            # pair's attention streams, and the exp stream never waits for
            # a whole-chunk backlog at qtile boundaries.
            AQ = [u["a"] for u in units]
            BQ = [[u["bq"][0], u["bkv"][0], u["bkv"][4], u["bkv"][5],
                   u["bq"][1], u["bkv"][1], u["bq"][2], u["bkv"][2],
                   u["bq"][3], u["bkv"][3]] for u in units]
            CUM = [4, 6, 8, 10]
            fpos = {}

            def flush(j, pair):
                ch = j // 2
                q = AQ[ch] if j % 2 == 0 else BQ[ch]
                key = (ch, j % 2)
                limit = CUM[pair]
                while fpos.get(key, 0) < limit:
                    q[fpos.get(key, 0)]()
                    fpos[key] = fpos.get(key, 0) + 1

            def extract_muls(j, pg, oav, opairs, on_act=False):
                # 1/d for 4 heads x 2 q-halves; all PSUM reads up front so
                # the oav slots release before any transpose allocates in
                # the shared "oav" psum tag rotation.
                for hf in range(2):
                    dinv = work.tile([128, 4], F32, tag="dinv", bufs=4,
                                     name=f"dinv{j}_{pg}_{hf}")
                    nc.vector.reciprocal(
                        dinv,
                        oav[hf].rearrange(
                            "p (s c) -> p s c", c=128)[:, :, 64:65],
                    )
                    for pi, pr in enumerate((2 * pg, 2 * pg + 1)):
                        opair = work.tile([128, 128], BF16, tag="opair",
                                          bufs=5, name=f"op{j}_{hf}_{pr}")
                        for s in range(2):
                            sl = (2 * pr + s) % 4
                            if on_act:
                                nc.scalar.activation(
                                    opair[:, s * 64:(s + 1) * 64],
                                    oav[hf][:, sl * 128:sl * 128 + 64],
                                    mybir.ActivationFunctionType.Identity,
                                    scale=dinv[:, sl:sl + 1],
                                )
                            else:
                                nc.vector.tensor_scalar_mul(
                                    opair[:, s * 64:(s + 1) * 64],
                                    oav[hf][:, sl * 128:sl * 128 + 64],
                                    dinv[:, sl:sl + 1],
                                )
                        opairs[hf][pi] = opair

            def extract_tr(j, pg, hf, opairs, o_j):
                for pi, pr in enumerate((2 * pg, 2 * pg + 1)):
                    ot = psum.tile([128, 128], BF16, tag="oav", bufs=2,
                                   name=f"ot{j}_{hf}_{pr}")
                    nc.tensor.transpose(ot, opairs[hf][pi], ident)
                    nc.vector.tensor_copy(
                        o_j[pr][:, hf * 128:(hf + 1) * 128], ot
                    )

            def outproj(j, mm, o_j, tail=False, ptag="oav"):
                jq = j * QT
                ysb = work.tile([128, C], F32, tag="ysb", bufs=3,
                                name=f"ys{j}_{mm}")
                for nn in range(2):
                    psy = psum.tile([128, 512], F32, tag=ptag, bufs=2,
                                    name=f"py{j}_{mm}_{nn}")
                    for p in range(4):
                        nc.tensor.matmul(
                            psy,
                            o_j[p][:, mm * 128:(mm + 1) * 128],
                            wp_sb[p][:, nn * 512:(nn + 1) * 512],
                            start=(p == 0),
                            stop=(p == 3),
                        )
                    if tail and nn == 1:
                        nc.scalar.activation(
                            ysb[:, nn * 512:(nn + 1) * 512], psy,
                            mybir.ActivationFunctionType.Identity)
                    else:
                        nc.vector.tensor_copy(
                            ysb[:, nn * 512:(nn + 1) * 512], psy)
                    nc.sync.dma_start(
                        y[jq + mm * 128:jq + (mm + 1) * 128,
                          nn * 512:(nn + 1) * 512],
                        ysb[:, nn * 512:(nn + 1) * 512],
                    )

            # qtile 0 pair 0/1's dependencies run before any attention
            flush(0, 1)
            deferred = []

            for j in range(8):
                ch = j // 2
                jq = j * QT
                ntk = 2 * (j + 1)
                ngrp = (ntk + 3) // 4
                o_j = [
                    work.tile([128, QT], BF16, tag=f"oj{p}", bufs=6,
                              name=f"o{p}_{j}")
                    for p in range(4)
                ]
                opairs = [[None, None], [None, None]]
                for pg in range(2):
                    oav = [
                        psum.tile([128, 512], F32, tag="oav", bufs=2,
                                  name=f"oav{j}_{pg}_{hf}")
                        for hf in range(2)
                    ]
                    for pair in (2 * pg, 2 * pg + 1):
                        for s in range(2):
                            h = 2 * pair + s
                            off = 64 * s
                            hslot = h % 4
                            pts = []
                            for g in range(ngrp):
                                blk = min(4, ntk - 4 * g)
                                diag = g == j // 2
                                # last key tile 2j+1: queries 0:127 are fully
                                # masked - compute only the valid q half
                                cols = blk * QT - (128 if diag else 0)
                                sg = psum.tile([128, 1024], F32, tag="sg",
                                               bufs=2, name=f"sg{j}_{h}_{g}")
                                for bi in range(blk):
                                    i = 4 * g + bi
                                    if diag and i == ntk - 1:
                                        nc.tensor.matmul(
                                            sg[:, bi * QT:bi * QT + 128],
                                            k_sb[pair][off:off + 64,
                                                       i * 128:(i + 1) * 128],
                                            q_sb[pair][off:off + 64,
                                                       jq + 128:jq + QT],
                                            start=True,
                                            stop=True,
                                        )
                                    else:
                                        nc.tensor.matmul(
                                            sg[:, bi * QT:(bi + 1) * QT],
                                            k_sb[pair][off:off + 64,
                                                       i * 128:(i + 1) * 128],
                                            q_sb[pair][off:off + 64, jq:jq + QT],
                                            start=True,
                                            stop=True,
                                        )
                                pt = work.tile([128, 1024], BF16, tag="pt",
                                               bufs=5, name=f"pt{j}_{h}_{g}")
                                nc.scalar.activation(
                                    pt[:, :cols], sg[:, :cols], Exp,
                                    scale=0.125)
                                if diag:  # diagonal key tiles 2j, 2j+1
                                    pos = (blk - 2) * QT
                                    nc.gpsimd.tensor_mul(
                                        pt[:, pos:pos + 384],
                                        pt[:, pos:pos + 384], mask_b)
                                pts.append(pt)
                                for bi in range(blk):  # q half 0
                                    i = 4 * g + bi
                                    if i == ntk - 1:
                                        continue  # fully masked for q half 0
                                    nc.tensor.matmul(
                                        oav[0][:, hslot * 128:hslot * 128 + 65],
                                        pt[:, bi * QT:bi * QT + 128],
                                        vnat[i][:, h * 65:h * 65 + 65],
                                        start=(i == 0),
                                        stop=(i == ntk - 2),
                                    )
                            for g in range(ngrp):  # q half 1
                                blk = min(4, ntk - 4 * g)
                                for bi in range(blk):
                                    i = 4 * g + bi
                                    lo = bi * QT + (0 if i == ntk - 1 else 128)
                                    nc.tensor.matmul(
                                        oav[1][:, hslot * 128:hslot * 128 + 65],
                                        pts[g][:, lo:lo + 128],
                                        vnat[i][:, h * 65:h * 65 + 65],
                                        start=(i == 0),
                                        stop=(i == ntk - 1),
                                    )
                        if pair < 3:
                            flush(j, pair + 1)
                        elif j < 7:
                            flush(j + 1, 1)
                    extract_muls(j, pg, oav, opairs,
                                 on_act=(j == 7 and pg == 1))
                    if pg == 0:
                        extract_tr(j, 0, 0, opairs, o_j)
                        extract_tr(j, 0, 1, opairs, o_j)
                if j in (2, 3, 4, 5, 6):
                    # defer this qtile's output projection into the next
                    # qtile's exp-heavy window as extra PE filler
                    extract_tr(j, 1, 0, opairs, o_j)
                    extract_tr(j, 1, 1, opairs, o_j)
                    deferred.append((j, o_j))
                else:
                    # interleave last extractions with the output projection
                    extract_tr(j, 1, 0, opairs, o_j)
                    outproj(j, 0, o_j, tail=(j == 7))
                    extract_tr(j, 1, 1, opairs, o_j)
                    outproj(j, 1, o_j, tail=(j == 7))
                for dj, do_j in list(deferred):
                    if (j == 6 and dj in (4, 5)) or (j == 7 and dj in (2, 3, 6)):
                        deferred.remove((dj, do_j))
                        outproj(dj, 0, do_j, tail=(j == 7), ptag="ps")
                        outproj(dj, 1, do_j, tail=(j == 7), ptag="ps")

    nc.finalize()
    return nc


_NC = None


def _get_nc():
    global _NC
    if _NC is None:
        _NC = build_kernel()
    return _NC


def kernel(x, Wqkv, bqkv, Wproj, bproj, _trace=False):
    x = np.asarray(x, dtype=np.float32)
    Wqkv = np.asarray(Wqkv, dtype=np.float32)
    bqkv = np.asarray(bqkv, dtype=np.float32)
    Wproj = np.asarray(Wproj, dtype=np.float32)
    bproj = np.asarray(bproj, dtype=np.float32)

    tri = np.triu(np.ones((2 * QT, 2 * QT), dtype=np.float32))[:, :QT]
    mask = np.ascontiguousarray(np.concatenate(
        [tri[0:128, 0:QT], tri[128:256, 128:QT]], axis=1)).astype(BF)
    in_maps = []
    for hg in range(2):
        sl = slice(hg * HGF, (hg + 1) * HGF)
        rows = np.concatenate([
            Wqkv[sl],
            Wqkv[1024 + hg * HGF:1024 + (hg + 1) * HGF],
            Wqkv[2048 + hg * HGF:2048 + (hg + 1) * HGF],
        ])
        wqkvT_h = np.ascontiguousarray(rows.T).astype(BF)      # [C, 1536]
        bq = bqkv[sl].reshape(4, 128).T
        bk = bqkv[1024 + hg * HGF:1024 + (hg + 1) * HGF].reshape(4, 128).T
        bqk_h = np.ascontiguousarray(
            np.concatenate([bq, bk], axis=1), dtype=np.float32)  # [128, 8]
        wprojT_h = np.ascontiguousarray(Wproj[:, sl].T).astype(BF)  # [512, C]
        for b in range(B):
            in_maps.append(
                {
                    "xT": np.ascontiguousarray(x[b].T).astype(BF),
                    "wqkvT": wqkvT_h,
                    "bqk": bqk_h,
                    "wprojT": wprojT_h,
                    "mask01": mask,
                }
            )
    # core order: idx = hg * 4 + b
    res = run_bass_kernel_spmd(_get_nc(), in_maps, core_ids=list(range(8)),
                               trace=_trace)
    # v-bias contribution is linear: folded into one host-side constant
    yconst = (bproj + Wproj @ bqkv[2048:]).astype(np.float32)
    out = np.empty((B, T, C), dtype=np.float32)
    for b in range(B):
        out[b] = res.results[b]["y"] + res.results[4 + b]["y"] + yconst
    if _trace:
        return out, res
    return out


# revision 41
# speedup vs baseline: 1.0045x; 1.0037x over previous
"""Causal self-attention (B=4, T=2048, C=1024, NH=16) on 8 trn2 NeuronCores.

Sharding: core = (head_group hg in {0,1}) x (batch b in {0..3}).
Each core computes qkv projection + attention + partial output projection for
its 8 heads of its batch; host sums the two head-group partials per batch and
adds the output bias (plus the v-bias contribution, which is linear in Wproj
and therefore folded into a host-side constant: P(V + 1 bv^T)/d = PV/d + bv).

Layout strategy (all matmuls in bf16: 1 PE cycle/row at ANY moving size,
vs fp32r which needs >=256; rel err ~5e-3, well inside the 2e-2 gate):
  - x, Wqkv, Wproj are converted to bf16 on the host; everything stays
    resident in SBUF (no DRAM staging round-trips).
  - q, k computed transposed (qT/kT = W_slice @ x.T) so head_size lands on
    partitions for QK^T.  v is computed in NATURAL [token, feat] layout
    (lhsT = xT tile, rhs = Wv) so no PE transposes of V are needed.
  - S^T = K @ Q^T per (head, 4-keytile group) into one [128, 1024] PSUM
    2-bank tile; one ScalarE Exp per group (amortizes the ~190ns ACT per-op
    overhead); causal mask = one 0/1 multiply on GpSimd over the two
    diagonal key tiles (always within one group).
  - AV runs in natural orientation: out[128 q, 65] with rhs = [V_head | 1];
    the ones column makes the softmax denominator ride along as column 64
    (65 moving columns instead of 256 -> AV PE cost halves vs transposed).
    The two 128-query halves are processed sequentially per head so 8
    accumulators (4 aligned head slots x 128 cols) fit one PSUM bank each.
  - 1/d is applied as the PSUM->SBUF copy itself (per-partition
    tensor_scalar_mul), then one bf16 PE transpose per (head pair, q half)
    produces O^T for the row-parallel output projection.
  - The Tile scheduler is a greedy per-engine priority list scheduler
    (priority = emission order) and dependencies follow emission-order
    semantics (a read emitted before a write sees the old value).  The
    projection is therefore emitted as 256-column units flushed per-pair
    with one-pair lookahead inside the qtile that consumes them: each unit
    executes while the previous pair's attention streams, the exp conveyor
    never waits on a whole-chunk backlog at qtile boundaries, and in the
    exp(softmax)-heavy late qtiles the units are the dep-free PE filler.
    The output projections of qtiles 4-6 are deferred into qtiles 6/7
    (their only consumer is the output DMA) as additional late filler,
    with their PSUM tiles in the feed tag so they do not chain behind the
    attention pipeline's slot rotation.
  - PSUM budget (8 banks): sg (exp staging) 4, feed-proj "ps" 2, oav
    accumulators + transposes + out-proj "oav" 2.  Feed units get their own
    tag because tag slots are granted in emission order: sharing a tag with
    the attention pipeline would chain low-priority filler behind it.
"""

import sys

sys.path.insert(0, "/opt/trn_rl_repo")

import ml_dtypes
import numpy as np

import concourse.bacc as bacc
import concourse.bass as bass
import concourse.mybir as mybir
from concourse.bass_utils import run_bass_kernel_spmd
from concourse.masks import make_identity
from concourse.tile import TileContext

B, T, C, NH = 4, 2048, 1024, 16
HS = C // NH          # 64
HGF = 512             # features per head group (8 heads x 64)
QT = 256              # query tile
NKT = T // 128        # 16 key tiles
F32 = mybir.dt.float32
BF16 = mybir.dt.bfloat16
Exp = mybir.ActivationFunctionType.Exp
BF = ml_dtypes.bfloat16

KPOS = [1, 5, 7, 9]   # position of k_p within a 10-unit half-chunk feed


def build_kernel():
    nc = bacc.Bacc(None, target_bir_lowering=False)
    xT = nc.dram_tensor("xT", (C, T), BF16, kind="ExternalInput")
    wqkvT = nc.dram_tensor("wqkvT", (C, 3 * HGF), BF16, kind="ExternalInput")
    bqk = nc.dram_tensor("bqk", (128, 8), F32, kind="ExternalInput")
    wprojT = nc.dram_tensor("wprojT", (HGF, C), BF16, kind="ExternalInput")
    mask01 = nc.dram_tensor("mask01", (128, 384), BF16, kind="ExternalInput")
    y = nc.dram_tensor("y", (T, C), F32, kind="ExternalOutput")

    with TileContext(nc) as tc:
        with (
            tc.tile_pool(name="outer", bufs=1) as outer,
            tc.tile_pool(name="work", bufs=1) as work,
            tc.tile_pool(name="psum", bufs=1, space="PSUM") as psum,
        ):
            identf = outer.tile([128, 128], F32, name="identf")
            make_identity(nc, identf)
            ident = outer.tile([128, 128], BF16, name="ident")
            nc.vector.tensor_copy(ident, identf)

            x_all = outer.tile([128, 8 * T], BF16, name="x_all")
            w_all = outer.tile([128, 8 * 1536], BF16, name="w_all")
            q_sb = [outer.tile([128, T], BF16, name=f"q{p}") for p in range(4)]
            k_sb = [outer.tile([128, T], BF16, name=f"k{p}") for p in range(4)]
            # v natural [token 128, 8 heads x (64 data + 1 ones)]
            vnat = [outer.tile([128, 520], BF16, name=f"v{i}") for i in range(NKT)]
            wp_sb = [outer.tile([128, C], BF16, name=f"wp{p}") for p in range(4)]
            mask_b = outer.tile([128, 384], BF16, name="mask_b")
            bqk_sb = outer.tile([128, 8], F32, name="bqk_sb")

            # denominator ones columns; v copies only touch the data columns
            for i in range(NKT):
                nc.vector.memset(
                    vnat[i].rearrange("p (h c) -> p h c", c=65)[:, :, 64:65], 1.0
                )

            # loads as wide single-DMA waves (HWDGE + the DMA engine pool
            # are single-slot in the cost model: few big transfers beat many
            # small ones).  Column order = first need: w q-cols, x chunk 0,
            # w k-cols, w v-cols, then the remaining x chunks.
            w_dst = w_all.rearrange("p (k f) -> p k f", f=1536)
            w_src = wqkvT.rearrange("(k p) f -> p k f", p=128)
            x_dst = x_all.rearrange("p (k t) -> p k t", t=T)
            x_src = xT.rearrange("(k p) t -> p k t", p=128)
            nc.sync.dma_start(w_dst[:, :, 0:128], w_src[:, :, 0:128])
            nc.scalar.dma_start(x_dst[:, :, 0:256], x_src[:, :, 0:256])
            nc.gpsimd.dma_start(bqk_sb, bqk[:, :])
            nc.gpsimd.dma_start(mask_b, mask01[:, :])
            nc.sync.dma_start(w_dst[:, :, 512:640], w_src[:, :, 512:640])
            nc.scalar.dma_start(w_dst[:, :, 1024:1536], w_src[:, :, 1024:1536])
            nc.scalar.dma_start(x_dst[:, :, 256:512], x_src[:, :, 256:512])
            nc.sync.dma_start(w_dst[:, :, 128:512], w_src[:, :, 128:512])
            nc.sync.dma_start(w_dst[:, :, 640:1024], w_src[:, :, 640:1024])
            nc.sync.dma_start(x_dst[:, :, 512:1024], x_src[:, :, 512:1024])
            nc.scalar.dma_start(x_dst[:, :, 1024:1536], x_src[:, :, 1024:1536])
            nc.sync.dma_start(x_dst[:, :, 1536:2048], x_src[:, :, 1536:2048])
            for p in range(4):
                (nc.gpsimd if p % 2 == 0 else nc.scalar).dma_start(
                    wp_sb[p], wprojT[p * 128:(p + 1) * 128, :])

            def emit_qk(n, m, half):
                c0 = n * 512 + half * 256
                ps = psum.tile([128, 256], F32, tag="ps", bufs=2,
                               name=f"ps{n}_{m}_{half}")
                for k in range(8):
                    nc.tensor.matmul(
                        ps,
                        w_all[:, k * 1536 + m * 128:k * 1536 + (m + 1) * 128],
                        x_all[:, k * T + c0:k * T + c0 + 256],
                        start=(k == 0),
                        stop=(k == 7),
                    )
                dst = q_sb[m] if m < 4 else k_sb[m - 4]
                nc.vector.tensor_scalar_add(
                    dst[:, c0:c0 + 256], ps, bqk_sb[:, m:m + 1]
                )

            def emit_v(n, t4):
                tk = 4 * n + t4
                psv = psum.tile([128, 512], F32, tag="ps", bufs=2,
                                name=f"psv{tk}")
                for k in range(8):
                    nc.tensor.matmul(
                        psv,
                        x_all[:, k * T + tk * 128:k * T + (tk + 1) * 128],
                        w_all[:, k * 1536 + 1024:(k + 1) * 1536],
                        start=(k == 0),
                        stop=(k == 7),
                    )
                nc.vector.tensor_copy(
                    vnat[tk].rearrange("p (h c) -> p h c", c=65)[:, :, 0:64],
                    psv.rearrange("p (h c) -> p h c", c=64),
                )

            # feed units per chunk: a = first 256 tokens (needed by
            # qtile 2c), bq = q of second 256 (needed at qtile 2c+1 start),
            # bkv = k/v of second 256 (needed only by qtile 2c+1's LAST
            # QK/AV groups - legal to emit at lowest priority so the list
            # scheduler pulls them into PE's exp-wait stalls).
            units = []
            for n in range(4):
                a, bq, bkv = [], [], []
                for p in range(4):
                    a.append(lambda n=n, m=p: emit_qk(n, m, 0))
                    a.append(lambda n=n, m=p: emit_qk(n, 4 + m, 0))
                    bq.append(lambda n=n, m=p: emit_qk(n, m, 1))
                    bkv.append(lambda n=n, m=p: emit_qk(n, 4 + m, 1))
                a.insert(2, lambda n=n: emit_v(n, 0))
                a.insert(3, lambda n=n: emit_v(n, 1))
                bkv.append(lambda n=n: emit_v(n, 2))
                bkv.append(lambda n=n: emit_v(n, 3))
                units.append({"a": a, "bq": bq, "bkv": bkv})
            # per-chunk queues in pair-need order; flushed per-pair with
            # one-pair lookahead so each unit executes while the previous
            # pair's attention streams, and the exp stream never waits for
            # a whole-chunk backlog at qtile boundaries.
            AQ = [u["a"] for u in units]
            BQ = [[u["bq"][0], u["bkv"][0], u["bkv"][4], u["bkv"][5],
                   u["bq"][1], u["bkv"][1], u["bq"][2], u["bkv"][2],
                   u["bq"][3], u["bkv"][3]] for u in units]
            CUM = [4, 6, 8, 10]
            fpos = {}

            def flush(j, pair):
                ch = j // 2
                q = AQ[ch] if j % 2 == 0 else BQ[ch]
                key = (ch, j % 2)
                limit = CUM[pair]
                while fpos.get(key, 0) < limit:
                    q[fpos.get(key, 0)]()
                    fpos[key] = fpos.get(key, 0) + 1

            def extract_muls(j, pg, oav, opairs, on_act=False):
                # 1/d for 4 heads x 2 q-halves; all PSUM reads up front so
                # the oav slots release before any transpose allocates in
                # the shared "oav" psum tag rotation.
                for hf in range(2):
                    dinv = work.tile([128, 4], F32, tag="dinv", bufs=4,
                                     name=f"dinv{j}_{pg}_{hf}")
                    nc.vector.reciprocal(
                        dinv,
                        oav[hf].rearrange(
                            "p (s c) -> p s c", c=128)[:, :, 64:65],
                    )
                    for pi, pr in enumerate((2 * pg, 2 * pg + 1)):
                        opair = work.tile([128, 128], BF16, tag="opair",
                                          bufs=5, name=f"op{j}_{hf}_{pr}")
                        for s in range(2):
                            sl = (2 * pr + s) % 4
                            if on_act:
                                nc.scalar.activation(
                                    opair[:, s * 64:(s + 1) * 64],
                                    oav[hf][:, sl * 128:sl * 128 + 64],
                                    mybir.ActivationFunctionType.Identity,
                                    scale=dinv[:, sl:sl + 1],
                                )
                            else:
                                nc.vector.tensor_scalar_mul(
                                    opair[:, s * 64:(s + 1) * 64],
                                    oav[hf][:, sl * 128:sl * 128 + 64],
                                    dinv[:, sl:sl + 1],
                                )
                        opairs[hf][pi] = opair

            def extract_tr(j, pg, hf, opairs, o_j):
                for pi, pr in enumerate((2 * pg, 2 * pg + 1)):
                    ot = psum.tile([128, 128], BF16, tag="oav", bufs=2,
                                   name=f"ot{j}_{hf}_{pr}")
                    nc.tensor.transpose(ot, opairs[hf][pi], ident)
                    nc.vector.tensor_copy(
                        o_j[pr][:, hf * 128:(hf + 1) * 128], ot
                    )

            def outproj(j, mm, o_j, tail=False, ptag="oav"):
                jq = j * QT
                ysb = work.tile([128, C], F32, tag="ysb", bufs=3,
                                name=f"ys{j}_{mm}")
                for nn in range(2):
                    psy = psum.tile([128, 512], F32, tag=ptag, bufs=2,
                                    name=f"py{j}_{mm}_{nn}")
                    for p in range(4):
                        nc.tensor.matmul(
                            psy,
                            o_j[p][:, mm * 128:(mm + 1) * 128],
                            wp_sb[p][:, nn * 512:(nn + 1) * 512],
                            start=(p == 0),
                            stop=(p == 3),
                        )
                    if tail and nn == 1:
                        nc.scalar.activation(
                            ysb[:, nn * 512:(nn + 1) * 512], psy,
                            mybir.ActivationFunctionType.Identity)
                    else:
                        nc.vector.tensor_copy(
                            ysb[:, nn * 512:(nn + 1) * 512], psy)
                    nc.sync.dma_start(
                        y[jq + mm * 128:jq + (mm + 1) * 128,
                          nn * 512:(nn + 1) * 512],
                        ysb[:, nn * 512:(nn + 1) * 512],
                    )

            # qtile 0 pair 0/1's dependencies run before any attention
            flush(0, 1)
            deferred = []

            for j in range(8):
                ch = j // 2
                jq = j * QT
                ntk = 2 * (j + 1)
                ngrp = (ntk + 3) // 4
                o_j = [
                    work.tile([128, QT], BF16, tag=f"oj{p}", bufs=6,
                              name=f"o{p}_{j}")
                    for p in range(4)
                ]
                opairs = [[None, None], [None, None]]
                for pg in range(2):
                    oav = [
                        psum.tile([128, 512], F32, tag="oav", bufs=2,
                                  name=f"oav{j}_{pg}_{hf}")
                        for hf in range(2)
                    ]
                    for pair in (2 * pg, 2 * pg + 1):
                        for s in range(2):
                            h = 2 * pair + s
                            off = 64 * s
                            hslot = h % 4
                            pts = []
                            for g in range(ngrp):
                                blk = min(4, ntk - 4 * g)
                                diag = g == j // 2
                                # last key tile 2j+1: queries 0:127 are fully
                                # masked - compute only the valid q half
                                cols = blk * QT - (128 if diag else 0)
                                sg = psum.tile([128, 1024], F32, tag="sg",
                                               bufs=2, name=f"sg{j}_{h}_{g}")
                                for bi in range(blk):
                                    i = 4 * g + bi
                                    if diag and i == ntk - 1:
                                        nc.tensor.matmul(
                                            sg[:, bi * QT:bi * QT + 128],
                                            k_sb[pair][off:off + 64,
                                                       i * 128:(i + 1) * 128],
                                            q_sb[pair][off:off + 64,
                                                       jq + 128:jq + QT],
                                            start=True,
                                            stop=True,
                                        )
                                    else:
                                        nc.tensor.matmul(
                                            sg[:, bi * QT:(bi + 1) * QT],
                                            k_sb[pair][off:off + 64,
                                                       i * 128:(i + 1) * 128],
                                            q_sb[pair][off:off + 64, jq:jq + QT],
                                            start=True,
                                            stop=True,
                                        )
                                pt = work.tile([128, 1024], BF16, tag="pt",
                                               bufs=5, name=f"pt{j}_{h}_{g}")
                                nc.scalar.activation(
                                    pt[:, :cols], sg[:, :cols], Exp,
                                    scale=0.125)
                                if diag:  # diagonal key tiles 2j, 2j+1
                                    pos = (blk - 2) * QT
                                    nc.gpsimd.tensor_mul(
                                        pt[:, pos:pos + 384],
                                        pt[:, pos:pos + 384], mask_b)
                                pts.append(pt)
                                for bi in range(blk):  # q half 0
                                    i = 4 * g + bi
                                    if i == ntk - 1:
                                        continue  # fully masked for q half 0
                                    nc.tensor.matmul(
                                        oav[0][:, hslot * 128:hslot * 128 + 65],
                                        pt[:, bi * QT:bi * QT + 128],
                                        vnat[i][:, h * 65:h * 65 + 65],
                                        start=(i == 0),
                                        stop=(i == ntk - 2),
                                    )
                            for g in range(ngrp):  # q half 1
                                blk = min(4, ntk - 4 * g)
                                for bi in range(blk):
                                    i = 4 * g + bi
                                    lo = bi * QT + (0 if i == ntk - 1 else 128)
                                    nc.tensor.matmul(
                                        oav[1][:, hslot * 128:hslot * 128 + 65],
                                        pts[g][:, lo:lo + 128],
                                        vnat[i][:, h * 65:h * 65 + 65],
                                        start=(i == 0),
                                        stop=(i == ntk - 1),
                                    )
                        if pair < 3:
                            flush(j, pair + 1)
                        elif j < 7:
                            flush(j + 1, 1)
                    extract_muls(j, pg, oav, opairs,
                                 on_act=(j == 7 and pg == 1))
                    if pg == 0:
                        extract_tr(j, 0, 0, opairs, o_j)
                        extract_tr(j, 0, 1, opairs, o_j)
                if j in (2, 3, 4, 5, 6):
                    # defer this qtile's output projection into the next
                    # qtile's exp-heavy window as extra PE filler
                    extract_tr(j, 1, 0, opairs, o_j)
                    extract_tr(j, 1, 1, opairs, o_j)
                    deferred.append((j, o_j))
                else:
                    # interleave last extractions with the output projection
                    extract_tr(j, 1, 0, opairs, o_j)
                    outproj(j, 0, o_j, tail=(j == 7))
                    extract_tr(j, 1, 1, opairs, o_j)
                    outproj(j, 1, o_j, tail=(j == 7))
                for dj, do_j in list(deferred):
                    if (j == 6 and dj in (2, 3, 4, 5)) or (j == 7 and dj == 6):
                        deferred.remove((dj, do_j))
                        outproj(dj, 0, do_j, tail=(j == 7), ptag="ps")
                        outproj(dj, 1, do_j, tail=(j == 7), ptag="ps")

    nc.finalize()
    return nc


_NC = None


def _get_nc():
    global _NC
    if _NC is None:
        _NC = build_kernel()
    return _NC


def kernel(x, Wqkv, bqkv, Wproj, bproj, _trace=False):
    x = np.asarray(x, dtype=np.float32)
    Wqkv = np.asarray(Wqkv, dtype=np.float32)
    bqkv = np.asarray(bqkv, dtype=np.float32)
    Wproj = np.asarray(Wproj, dtype=np.float32)
    bproj = np.asarray(bproj, dtype=np.float32)

    tri = np.triu(np.ones((2 * QT, 2 * QT), dtype=np.float32))[:, :QT]
    mask = np.ascontiguousarray(np.concatenate(
        [tri[0:128, 0:QT], tri[128:256, 128:QT]], axis=1)).astype(BF)
    in_maps = []
    for hg in range(2):
        sl = slice(hg * HGF, (hg + 1) * HGF)
        rows = np.concatenate([
            Wqkv[sl],
            Wqkv[1024 + hg * HGF:1024 + (hg + 1) * HGF],
            Wqkv[2048 + hg * HGF:2048 + (hg + 1) * HGF],
        ])
        wqkvT_h = np.ascontiguousarray(rows.T).astype(BF)      # [C, 1536]
        bq = bqkv[sl].reshape(4, 128).T
        bk = bqkv[1024 + hg * HGF:1024 + (hg + 1) * HGF].reshape(4, 128).T
        bqk_h = np.ascontiguousarray(
            np.concatenate([bq, bk], axis=1), dtype=np.float32)  # [128, 8]
        wprojT_h = np.ascontiguousarray(Wproj[:, sl].T).astype(BF)  # [512, C]
        for b in range(B):
            in_maps.append(
                {
                    "xT": np.ascontiguousarray(x[b].T).astype(BF),
                    "wqkvT": wqkvT_h,
                    "bqk": bqk_h,
                    "wprojT": wprojT_h,
                    "mask01": mask,
                }
            )
    # core order: idx = hg * 4 + b
    res = run_bass_kernel_spmd(_get_nc(), in_maps, core_ids=list(range(8)),
                               trace=_trace)
    # v-bias contribution is linear: folded into one host-side constant
    yconst = (bproj + Wproj @ bqkv[2048:]).astype(np.float32)
    out = np.empty((B, T, C), dtype=np.float32)
    for b in range(B):
        out[b] = res.results[b]["y"] + res.results[4 + b]["y"] + yconst
    if _trace:
        return out, res
    return out
